# revision 10
# baseline (speedup 1.0000x reference)
"""DeepFM Bass/Tile kernel for TRN2 — v3.

Differences from v2:
- Gathers use the [128,1] per-partition-index indirect-DMA pattern (the only
  one that matches this runtime's ucode; [128,K>1] reads indices from wrong
  partitions on HW). 32 calls per 2048-sample macro, one 65-float row per
  partition per call, from a fused table tab[200000, 65] with asymmetric row
  layouts: user rows = [lin_w | emb], item rows = [emb | lin_w]. A slot then
  holds [u_lin, u_emb(64), i_emb(64), i_lin] so cols 1..128 are the
  contiguous 128-feature vector for one [128,128] partition-0 transpose.
- Sample order within a macro: m = 16*p + k (p=partition, k=0..15 slot).
- Cross term via C = (K K^T - diag(rowsum K^2))/256:
  cross[b] = x C x^T = ones^T (x ⊙ (C x)), folded into the same
  data-stationary scalar matmuls as the DNN head. No Square op, no
  second cross matmul.
"""

from contextlib import ExitStack

import numpy as np

import concourse.bass as bass
import concourse.tile as tile
from concourse import bacc, mybir
from concourse.masks import make_identity

FP32 = mybir.dt.float32
BF16 = mybir.dt.bfloat16
I32 = mybir.dt.int32
AX = mybir.AluOpType
AF = mybir.ActivationFunctionType

HASH_BINS = 100000
EMB = 64
TABW = 65  # 64 emb floats + 1 lin_w float
TILE = 512
MACRO = 2048
K_PER_P = MACRO // 128  # 16


def _fv(t_ap: bass.AP, start: int, step: int, count: int) -> bass.AP:
    """[128, X] tile -> strided free view [128, count] (elements start::step)."""
    part_dim = t_ap.ap[0]
    return bass.AP(t_ap.tensor, t_ap.offset + start, [part_dim, [step, count]])


def emit_dfm(ctx: ExitStack, tc: tile.TileContext, aps: dict, S: int, passes: int = 1):
    nc = tc.nc
    n_macros = S // MACRO
    assert S % MACRO == 0

    const_p = ctx.enter_context(tc.tile_pool(name="const", bufs=1))
    idx_p = ctx.enter_context(tc.tile_pool(name="idx", bufs=5))
    g_p = ctx.enter_context(tc.tile_pool(name="gath", bufs=5))
    act_p = ctx.enter_context(tc.tile_pool(name="act", bufs=3))
    out_p = ctx.enter_context(tc.tile_pool(name="out", bufs=3))
    ps_a = ctx.enter_context(tc.tile_pool(name="ps_a", bufs=4, space="PSUM"))
    ps_b = ctx.enter_context(tc.tile_pool(name="ps_b", bufs=3, space="PSUM"))
    ps_s = ctx.enter_context(tc.tile_pool(name="ps_s", bufs=1, space="PSUM"))

    ident = const_p.tile([128, 128], FP32, tag="ident")
    make_identity(nc, ident[:])

    def load_sb(name, src_ap, shape, dtype=FP32):
        t = const_p.tile(shape, dtype, tag=name, name=name)
        nc.sync.dma_start(t[:], src_ap)
        return t

    tab = aps["tab"]  # [200000, 65] fused emb|lin table

    # ---- warmup gathers: first indirect call on this runtime is flaky ----
    wk_idx = const_p.tile([128, 1], I32, tag="wk_idx")
    nc.vector.memset(wk_idx[:], 0)
    wk_out = const_p.tile([128, TABW], FP32, tag="wk_out")
    for _ in range(2):
        nc.gpsimd.indirect_dma_start(
            out=wk_out[:, :],
            out_offset=None,
            in_=tab[:, :],
            in_offset=bass.IndirectOffsetOnAxis(ap=wk_idx[:, :], axis=0),
        )

    # ---- constants ----
    k_f32 = load_sb("k_f32", aps["cross_k"][:, :], [128, 128])
    k_bf = const_p.tile([128, 128], BF16, tag="k_bf")
    nc.vector.tensor_copy(k_bf[:], k_f32[:])
    # K^T via PE transpose
    kt_ps = ps_a.tile([128, 128], FP32, tag="pa", name="kt_ps")
    nc.tensor.transpose(out=kt_ps[:], in_=k_f32[:], identity=ident[:])
    kt_bf = const_p.tile([128, 128], BF16, tag="kt_bf")
    nc.vector.tensor_copy(kt_bf[:], kt_ps[:])
    # A = K K^T = (K^T)^T @ (K^T)
    a_ps = ps_b.tile([128, 128], FP32, tag="pb", name="a_ps")
    nc.tensor.matmul(out=a_ps[:], lhsT=kt_bf[:], rhs=kt_bf[:], start=True, stop=True)
    # R = rowsum(K^2)
    k_sq = const_p.tile([128, 128], FP32, tag="k_sq")
    nc.scalar.activation(k_sq[:], k_f32[:], AF.Square)
    r_vec = const_p.tile([128, 1], FP32, tag="r_vec")
    nc.vector.reduce_sum(out=r_vec[:], in_=k_sq[:], axis=mybir.AxisListType.X)
    # C = (A - diag(R)) / 256  (bf16)
    diagr = const_p.tile([128, 128], FP32, tag="diagr")
    nc.vector.tensor_scalar(
        out=diagr[:], in0=ident[:], scalar1=r_vec[:], scalar2=None, op0=AX.mult
    )
    c_f32 = const_p.tile([128, 128], FP32, tag="c_f32")
    nc.vector.tensor_tensor(out=c_f32[:], in0=a_ps[:], in1=diagr[:], op=AX.subtract)
    c_bf = const_p.tile([128, 128], BF16, tag="c_bf")
    nc.vector.tensor_scalar(
        out=c_bf[:], in0=c_f32[:], scalar1=1.0 / 256.0, scalar2=None, op0=AX.mult
    )

    ones1 = const_p.tile([128, 1], BF16, tag="ones1")
    nc.vector.memset(ones1[:], 1.0)
    off_h = const_p.tile([128, K_PER_P], I32, tag="off_h")
    nc.vector.memset(off_h[:], HASH_BINS)

    def wchunks(name, K, M):
        src = aps[name]
        chunks = []
        for ki, k0 in enumerate(range(0, K, 128)):
            for mi, m0 in enumerate(range(0, M, 128)):
                kk = min(128, K - k0)
                mm = min(128, M - m0)
                f = load_sb(
                    f"{name}_f{ki}{mi}", src[k0 : k0 + kk, m0 : m0 + mm], [kk, mm]
                )
                b = const_p.tile([kk, mm], BF16, tag=f"{name}_b{ki}{mi}")
                nc.vector.tensor_copy(b[:], f[:])
                chunks.append((ki, mi, kk, mm, b))
        return chunks

    W1c = wchunks("W1", 128, 200)
    W2c = wchunks("W2", 200, 256)
    W3c = wchunks("W3", 256, 200)
    W4c = wchunks("W4", 200, 128)
    W5f = load_sb("W5_f", aps["W5"][:, :], [128, 1])
    W5b = const_p.tile([128, 1], BF16, tag="W5_b")
    nc.vector.tensor_copy(W5b[:], W5f[:])

    def bias_parts(name, D):
        src = aps[name]
        parts = []
        for k0 in range(0, D, 128):
            kk = min(128, D - k0)
            parts.append(load_sb(f"{name}_{k0}", src[k0 : k0 + kk], [kk, 1]))
        return parts

    b1p = bias_parts("b1", 200)
    b2p = bias_parts("b2", 256)
    b3p = bias_parts("b3", 200)
    b4p = bias_parts("b4", 128)

    linb = load_sb("linb", aps["lin_b"][:], [1, 1])
    b5s = load_sb("b5s", aps["b5"][:], [1, 1])
    c0 = const_p.tile([1, 1], FP32, tag="c0")
    nc.vector.tensor_tensor(out=c0[:], in0=linb[:], in1=b5s[:], op=AX.add)
    c0b = const_p.tile([128, 1], FP32, tag="c0b")
    nc.gpsimd.partition_broadcast(out_ap=c0b[:], in_ap=c0[:])

    u_idx, i_idx = aps["user_idx"], aps["item_idx"]
    y = aps["y"]

    # ---- software pipeline over all (macro, T) tiles -------------------
    # Stage A(t): transposes + embT copy + CX matmul + cx_sb + XCX.
    # Stage B(t): MLP + STB + arg. A is emitted LOOKAHEAD tiles ahead of B
    # so each engine's FIFO interleaves work from several tiles.
    LOOKAHEAD = 2
    macro_state: dict = {}

    def setup_macro(mt):
        s0 = mt * MACRO
        iu = idx_p.tile([128, K_PER_P], I32, tag="iu")
        nc.sync.dma_start(
            iu[:], bass.AP(u_idx.tensor, s0, [[K_PER_P, 128], [1, K_PER_P]])
        )
        ii = idx_p.tile([128, K_PER_P], I32, tag="ii")
        nc.sync.dma_start(
            ii[:], bass.AP(i_idx.tensor, s0, [[K_PER_P, 128], [1, K_PER_P]])
        )
        ii2 = idx_p.tile([128, K_PER_P], I32, tag="ii2")
        nc.vector.tensor_tensor(out=ii2[:], in0=ii[:], in1=off_h[:], op=AX.add)

        GU = g_p.tile([128, K_PER_P, 2 * TABW], FP32, tag="GU")
        for k in range(K_PER_P):
            nc.gpsimd.indirect_dma_start(
                out=GU[:, k, 0:TABW],
                out_offset=None,
                in_=tab[:, :],
                in_offset=bass.IndirectOffsetOnAxis(ap=iu[:, k : k + 1], axis=0),
            )
            nc.gpsimd.indirect_dma_start(
                out=GU[:, k, TABW : 2 * TABW],
                out_offset=None,
                in_=tab[:, :],
                in_offset=bass.IndirectOffsetOnAxis(ap=ii2[:, k : k + 1], axis=0),
            )

        LIN = g_p.tile([128, K_PER_P], FP32, tag="LIN")
        nc.vector.tensor_tensor(
            out=LIN[:],
            in0=_fv(GU[:, :, :], 0, 2 * TABW, K_PER_P),
            in1=_fv(GU[:, :, :], 2 * TABW - 1, 2 * TABW, K_PER_P),
            op=AX.add,
        )
        yt = out_p.tile([128, K_PER_P], FP32, tag="yt")
        argt = out_p.tile([128, K_PER_P], FP32, tag="argt")
        return {"GU": GU, "LIN": LIN, "yt": yt, "argt": argt, "s0": s0}

    def stage_a(st, T):
        gu_ap = st["GU"][:, :, :]
        ET = ps_a.tile([128, TILE], FP32, tag="pa", name="ET")
        for j in range(4):
            k = 4 * T + j
            pair_view = bass.AP(
                gu_ap.tensor,
                gu_ap.offset + k * 2 * TABW + 1,
                [gu_ap.ap[0], [1, 128]],
            )
            nc.tensor.transpose(
                out=ET[:, j * 128 : (j + 1) * 128],
                in_=pair_view,
                identity=ident[:],
            )
        embT = act_p.tile([128, TILE], BF16, tag="embT")
        nc.scalar.activation(embT[:], ET[:], AF.Copy)
        CX = ps_b.tile([128, TILE], FP32, tag="pb", name="CX")
        nc.tensor.matmul(out=CX[:], lhsT=c_bf[:], rhs=embT[:], start=True, stop=True)
        cx_sb = act_p.tile([128, TILE], BF16, tag="cx_sb")
        nc.scalar.activation(cx_sb[:], CX[:], AF.Copy)
        XCX = act_p.tile([128, TILE], BF16, tag="XCX")
        nc.vector.tensor_tensor(out=XCX[:], in0=embT[:], in1=cx_sb[:], op=AX.mult)
        return {"embT": embT, "XCX": XCX}

    def layer(rhs_parts, Wc, bp, Dout, tagb, relu_engines):
        outs = []
        n_m = (Dout + 127) // 128
        psums = []
        for mi in range(n_m):
            mm = min(128, Dout - mi * 128)
            if mi == 0:
                P = ps_a.tile([mm, TILE], FP32, tag="pa", name=f"{tagb}{mi}")
            else:
                P = ps_b.tile([mm, TILE], FP32, tag="pb", name=f"{tagb}{mi}")
            nk = len(rhs_parts)
            for ki in range(nk):
                Wt = next(c[4] for c in Wc if c[0] == ki and c[1] == mi)
                nc.tensor.matmul(
                    out=P[:],
                    lhsT=Wt[:],
                    rhs=rhs_parts[ki][:],
                    start=(ki == 0),
                    stop=(ki == nk - 1),
                )
            psums.append((mi, mm, P))
        for mi, mm, P in psums:
            o = act_p.tile([mm, TILE], BF16, tag=f"{tagb}o{mi}")
            if relu_engines[mi] == "act":
                nc.scalar.activation(o[:], P[:], AF.Relu, bias=bp[mi][:])
            else:
                nc.vector.tensor_scalar(
                    out=o[:],
                    in0=P[:],
                    scalar1=bp[mi][:],
                    scalar2=0.0,
                    op0=AX.add,
                    op1=AX.max,
                )
            outs.append(o)
        return outs

    def stage_b(st, T, a):
        embT, XCX = a["embT"], a["XCX"]
        h1 = layer([embT], W1c, b1p, 200, "H1", ("vec", "act"))
        h2 = layer(h1, W2c, b2p, 256, "H2", ("act", "vec"))
        h3 = layer(h2, W3c, b3p, 200, "H3", ("vec", "act"))
        h4 = layer(h3, W4c, b4p, 128, "H4", ("vec",))
        STB = ps_s.tile([128, 4], FP32, tag="ps", name="STB")
        for j in range(4):
            cj = slice(j * 128, (j + 1) * 128)
            nc.tensor.matmul(
                out=STB[:, j : j + 1], lhsT=XCX[:, cj], rhs=ones1[:],
                start=True, stop=False,
            )
            nc.tensor.matmul(
                out=STB[:, j : j + 1], lhsT=h4[0][:, cj], rhs=W5b[:],
                start=False, stop=True,
            )
        nc.vector.tensor_tensor(
            out=st["argt"][:, 4 * T : 4 * T + 4],
            in0=STB[:],
            in1=st["LIN"][:, 4 * T : 4 * T + 4],
            op=AX.add,
        )

    def finish_macro(st):
        nc.scalar.activation(st["yt"][:], st["argt"][:], AF.Sigmoid, bias=c0b[:])
        nc.sync.dma_start(
            bass.AP(y.tensor, st["s0"], [[K_PER_P, 128], [1, K_PER_P]]),
            st["yt"][:],
        )

    flat = [(i, T) for i in range(n_macros * passes) for T in range(4)]
    a_results: dict = {}
    for i in range(-LOOKAHEAD, len(flat)):
        if i + LOOKAHEAD < len(flat):
            ia, TA = flat[i + LOOKAHEAD]
            if ia not in macro_state:
                macro_state[ia] = setup_macro(ia % n_macros)
            a_results[(ia, TA)] = stage_a(macro_state[ia], TA)
        if i >= 0:
            ib, TB = flat[i]
            stage_b(macro_state[ib], TB, a_results.pop((ib, TB)))
            if TB == 3:
                finish_macro(macro_state.pop(ib))


def build_nc(
    S: int = 32768,
    num_devices: int = 8,
    enable_asserts: bool = False,
    passes: int = 1,
):
    nc = bacc.Bacc(
        "TRN2",
        target_bir_lowering=False,
        debug=False,
        enable_asserts=enable_asserts,
        num_devices=num_devices,
    )
    aps = {}
    specs = [
        ("user_idx", [S], I32),
        ("item_idx", [S], I32),
        ("tab", [2 * HASH_BINS, TABW], FP32),
        ("lin_b", [1], FP32),
        ("cross_k", [128, 128], FP32),
        ("W1", [128, 200], FP32),
        ("b1", [200], FP32),
        ("W2", [200, 256], FP32),
        ("b2", [256], FP32),
        ("W3", [256, 200], FP32),
        ("b3", [200], FP32),
        ("W4", [200, 128], FP32),
        ("b4", [128], FP32),
        ("W5", [128, 1], FP32),
        ("b5", [1], FP32),
    ]
    for name, shape, dt in specs:
        aps[name] = nc.dram_tensor(name, shape, dt, kind="ExternalInput").ap()
    aps["y"] = nc.dram_tensor("y", [S], FP32, kind="ExternalOutput").ap()

    with tile.TileContext(nc) as tc:
        with ExitStack() as ctx:
            emit_dfm(ctx, tc, aps, S, passes=passes)
    nc.compile()
    return nc


from concourse.bass_utils import run_bass_kernel_spmd

B_FULL = 262144
N_CORES = 8
S_CORE = B_FULL // N_CORES

_NC_CACHE: dict = {}


def make_in_maps(inputs: dict) -> list:
    ue = np.asarray(inputs["user_emb"], dtype=np.float32)
    ie = np.asarray(inputs["item_emb"], dtype=np.float32)
    lw = np.asarray(inputs["lin_w"], dtype=np.float32)
    tab = np.empty((2 * HASH_BINS, TABW), np.float32)
    tab[:HASH_BINS, 0] = lw[:HASH_BINS]        # user rows: [lin | emb]
    tab[:HASH_BINS, 1:] = ue
    tab[HASH_BINS:, :EMB] = ie                 # item rows: [emb | lin]
    tab[HASH_BINS:, EMB] = lw[HASH_BINS:]
    shared = {"tab": np.ascontiguousarray(tab)}
    for k in ("lin_b", "cross_k", "W1", "b1", "W2", "b2", "W3", "b3",
              "W4", "b4", "W5", "b5"):
        shared[k] = np.ascontiguousarray(np.asarray(inputs[k], dtype=np.float32))
    u = np.ascontiguousarray(np.asarray(inputs["user_idx"], dtype=np.int32))
    i = np.ascontiguousarray(np.asarray(inputs["item_idx"], dtype=np.int32))
    in_maps = []
    for c in range(N_CORES):
        m = dict(shared)
        m["user_idx"] = u[c * S_CORE : (c + 1) * S_CORE]
        m["item_idx"] = i[c * S_CORE : (c + 1) * S_CORE]
        in_maps.append(m)
    return in_maps


def kernel(**inputs) -> np.ndarray:
    if "nc" not in _NC_CACHE:
        _NC_CACHE["nc"] = build_nc(S_CORE, num_devices=N_CORES)
    nc = _NC_CACHE["nc"]
    res = run_bass_kernel_spmd(nc, make_in_maps(inputs), core_ids=list(range(N_CORES)))
    y = np.concatenate([res.results[c]["y"] for c in range(N_CORES)])
    return y.reshape(B_FULL, 1).astype(np.float32)


# revision 19
# speedup vs baseline: 1.4955x; 1.4955x over previous
"""DeepFM Bass/Tile kernel for TRN2 — v3.

Differences from v2:
- Gathers use the [128,1] per-partition-index indirect-DMA pattern (the only
  one that matches this runtime's ucode; [128,K>1] reads indices from wrong
  partitions on HW). 32 calls per 2048-sample macro, one 65-float row per
  partition per call, from a fused table tab[200000, 65] with asymmetric row
  layouts: user rows = [lin_w | emb], item rows = [emb | lin_w]. A slot then
  holds [u_lin, u_emb(64), i_emb(64), i_lin] so cols 1..128 are the
  contiguous 128-feature vector for one [128,128] partition-0 transpose.
- Sample order within a macro: m = 16*p + k (p=partition, k=0..15 slot).
- Cross term via C = (K K^T - diag(rowsum K^2))/256:
  cross[b] = x C x^T = ones^T (x ⊙ (C x)), folded into the same
  data-stationary scalar matmuls as the DNN head. No Square op, no
  second cross matmul.
"""

from contextlib import ExitStack

import numpy as np

import concourse.bass as bass
import concourse.tile as tile
from concourse import bacc, mybir
from concourse.masks import make_identity

FP32 = mybir.dt.float32
BF16 = mybir.dt.bfloat16
I32 = mybir.dt.int32
U16 = mybir.dt.uint16
AX = mybir.AluOpType
AF = mybir.ActivationFunctionType

HASH_BINS = 100000
EMB = 64
TABW = 65  # 64 emb floats + 1 lin_w float
TILE = 512
MACRO = 2048
K_PER_P = MACRO // 128  # 16


def _fv(t_ap: bass.AP, start: int, step: int, count: int) -> bass.AP:
    """[128, X] tile -> strided free view [128, count] (elements start::step)."""
    part_dim = t_ap.ap[0]
    return bass.AP(t_ap.tensor, t_ap.offset + start, [part_dim, [step, count]])


def emit_dfm(ctx: ExitStack, tc: tile.TileContext, aps: dict, S: int, passes: int = 1):
    nc = tc.nc
    n_macros = S // MACRO
    assert S % MACRO == 0

    const_p = ctx.enter_context(tc.tile_pool(name="const", bufs=1))
    idx_p = ctx.enter_context(tc.tile_pool(name="idx", bufs=5))
    g_p = ctx.enter_context(tc.tile_pool(name="gath", bufs=5))
    act_p = ctx.enter_context(tc.tile_pool(name="act", bufs=4))
    out_p = ctx.enter_context(tc.tile_pool(name="out", bufs=3))
    ps_a = ctx.enter_context(tc.tile_pool(name="ps_a", bufs=4, space="PSUM"))
    ps_b = ctx.enter_context(tc.tile_pool(name="ps_b", bufs=3, space="PSUM"))
    ps_s = ctx.enter_context(tc.tile_pool(name="ps_s", bufs=1, space="PSUM"))

    ident = const_p.tile([128, 128], FP32, tag="ident")
    make_identity(nc, ident[:])
    ident_bf = const_p.tile([128, 128], BF16, tag="ident_bf")
    nc.vector.tensor_copy(ident_bf[:], ident[:])

    def load_sb(name, src_ap, shape, dtype=FP32):
        t = const_p.tile(shape, dtype, tag=name, name=name)
        nc.sync.dma_start(t[:], src_ap)
        return t

    # [200000, 65] fused emb|lin table; input is uint16 bits of bf16
    tab = aps["tab"].bitcast(BF16)

    # ---- warmup gathers: first indirect call on this runtime is flaky ----
    wk_idx = const_p.tile([128, 1], I32, tag="wk_idx")
    nc.vector.memset(wk_idx[:], 0)
    wk_out = const_p.tile([128, TABW], BF16, tag="wk_out")
    for _ in range(2):
        nc.gpsimd.indirect_dma_start(
            out=wk_out[:, :],
            out_offset=None,
            in_=tab[:, :],
            in_offset=bass.IndirectOffsetOnAxis(ap=wk_idx[:, :], axis=0),
        )

    # ---- constants ----
    k_f32 = load_sb("k_f32", aps["cross_k"][:, :], [128, 128])
    k_bf = const_p.tile([128, 128], BF16, tag="k_bf")
    nc.vector.tensor_copy(k_bf[:], k_f32[:])
    # K^T via PE transpose
    kt_ps = ps_a.tile([128, 128], FP32, tag="pa", name="kt_ps")
    nc.tensor.transpose(out=kt_ps[:], in_=k_f32[:], identity=ident[:])
    kt_bf = const_p.tile([128, 128], BF16, tag="kt_bf")
    nc.vector.tensor_copy(kt_bf[:], kt_ps[:])
    # A = K K^T = (K^T)^T @ (K^T)
    a_ps = ps_b.tile([128, 128], FP32, tag="pb", name="a_ps")
    nc.tensor.matmul(out=a_ps[:], lhsT=kt_bf[:], rhs=kt_bf[:], start=True, stop=True)
    # R = rowsum(K^2)
    k_sq = const_p.tile([128, 128], FP32, tag="k_sq")
    nc.scalar.activation(k_sq[:], k_f32[:], AF.Square)
    r_vec = const_p.tile([128, 1], FP32, tag="r_vec")
    nc.vector.reduce_sum(out=r_vec[:], in_=k_sq[:], axis=mybir.AxisListType.X)
    # C = (A - diag(R)) / 256  (bf16)
    diagr = const_p.tile([128, 128], FP32, tag="diagr")
    nc.vector.tensor_scalar(
        out=diagr[:], in0=ident[:], scalar1=r_vec[:], scalar2=None, op0=AX.mult
    )
    c_f32 = const_p.tile([128, 128], FP32, tag="c_f32")
    nc.vector.tensor_tensor(out=c_f32[:], in0=a_ps[:], in1=diagr[:], op=AX.subtract)
    c_bf = const_p.tile([128, 128], BF16, tag="c_bf")
    nc.vector.tensor_scalar(
        out=c_bf[:], in0=c_f32[:], scalar1=1.0 / 256.0, scalar2=None, op0=AX.mult
    )

    ones1 = const_p.tile([128, 1], BF16, tag="ones1")
    nc.vector.memset(ones1[:], 1.0)
    off_h = const_p.tile([128, K_PER_P], I32, tag="off_h")
    nc.vector.memset(off_h[:], HASH_BINS)

    def wchunks(name, K, M):
        src = aps[name]
        chunks = []
        for ki, k0 in enumerate(range(0, K, 128)):
            for mi, m0 in enumerate(range(0, M, 128)):
                kk = min(128, K - k0)
                mm = min(128, M - m0)
                f = load_sb(
                    f"{name}_f{ki}{mi}", src[k0 : k0 + kk, m0 : m0 + mm], [kk, mm]
                )
                b = const_p.tile([kk, mm], BF16, tag=f"{name}_b{ki}{mi}")
                nc.vector.tensor_copy(b[:], f[:])
                chunks.append((ki, mi, kk, mm, b))
        return chunks

    W1c = wchunks("W1", 128, 200)
    W2c = wchunks("W2", 200, 256)
    W3c = wchunks("W3", 256, 200)
    W4c = wchunks("W4", 200, 128)
    W5f = load_sb("W5_f", aps["W5"][:, :], [128, 1])
    W5b = const_p.tile([128, 1], BF16, tag="W5_b")
    nc.vector.tensor_copy(W5b[:], W5f[:])

    def bias_parts(name, D):
        src = aps[name]
        parts = []
        for k0 in range(0, D, 128):
            kk = min(128, D - k0)
            parts.append(load_sb(f"{name}_{k0}", src[k0 : k0 + kk], [kk, 1]))
        return parts

    b1p = bias_parts("b1", 200)
    b2p = bias_parts("b2", 256)
    b3p = bias_parts("b3", 200)
    b4p = bias_parts("b4", 128)

    linb = load_sb("linb", aps["lin_b"][:], [1, 1])
    b5s = load_sb("b5s", aps["b5"][:], [1, 1])
    c0 = const_p.tile([1, 1], FP32, tag="c0")
    nc.vector.tensor_tensor(out=c0[:], in0=linb[:], in1=b5s[:], op=AX.add)
    c0b = const_p.tile([128, 1], FP32, tag="c0b")
    nc.gpsimd.partition_broadcast(out_ap=c0b[:], in_ap=c0[:])

    u_idx, i_idx = aps["user_idx"], aps["item_idx"]
    y = aps["y"]

    # ---- software pipeline over all (macro, T) tiles -------------------
    # Stage A(t): transposes + embT copy + CX matmul + cx_sb + XCX.
    # Stage B(t): MLP + STB + arg. A is emitted LOOKAHEAD tiles ahead of B
    # so each engine's FIFO interleaves work from several tiles.
    LOOKAHEAD = 0
    macro_state: dict = {}

    def setup_macro(mt):
        s0 = mt * MACRO
        iu = idx_p.tile([128, K_PER_P], I32, tag="iu")
        nc.sync.dma_start(
            iu[:], bass.AP(u_idx.tensor, s0, [[K_PER_P, 128], [1, K_PER_P]])
        )
        ii = idx_p.tile([128, K_PER_P], I32, tag="ii")
        nc.sync.dma_start(
            ii[:], bass.AP(i_idx.tensor, s0, [[K_PER_P, 128], [1, K_PER_P]])
        )
        ii2 = idx_p.tile([128, K_PER_P], I32, tag="ii2")
        nc.vector.tensor_tensor(out=ii2[:], in0=ii[:], in1=off_h[:], op=AX.add)

        GU = g_p.tile([128, K_PER_P, 2 * TABW], BF16, tag="GU")
        for k in range(K_PER_P):
            nc.gpsimd.indirect_dma_start(
                out=GU[:, k, 0:TABW],
                out_offset=None,
                in_=tab[:, :],
                in_offset=bass.IndirectOffsetOnAxis(ap=iu[:, k : k + 1], axis=0),
            )
            nc.gpsimd.indirect_dma_start(
                out=GU[:, k, TABW : 2 * TABW],
                out_offset=None,
                in_=tab[:, :],
                in_offset=bass.IndirectOffsetOnAxis(ap=ii2[:, k : k + 1], axis=0),
            )

        LIN = g_p.tile([128, K_PER_P], FP32, tag="LIN")
        nc.vector.tensor_tensor(
            out=LIN[:],
            in0=_fv(GU[:, :, :], 0, 2 * TABW, K_PER_P),
            in1=_fv(GU[:, :, :], 2 * TABW - 1, 2 * TABW, K_PER_P),
            op=AX.add,
        )
        yt = out_p.tile([128, K_PER_P], FP32, tag="yt")
        argt = out_p.tile([128, K_PER_P], FP32, tag="argt")
        return {"GU": GU, "LIN": LIN, "yt": yt, "argt": argt, "s0": s0}

    def stage_a(st, T):
        gu_ap = st["GU"][:, :, :]
        ET = ps_a.tile([128, TILE], BF16, tag="pa", name="ET")
        for j in range(4):
            k = 4 * T + j
            pair_view = bass.AP(
                gu_ap.tensor,
                gu_ap.offset + k * 2 * TABW + 1,
                [gu_ap.ap[0], [1, 128]],
            )
            nc.tensor.transpose(
                out=ET[:, j * 128 : (j + 1) * 128],
                in_=pair_view,
                identity=ident_bf[:],
            )
        embT = act_p.tile([128, TILE], BF16, tag="embT")
        nc.scalar.activation(embT[:], ET[:], AF.Copy)
        CX = ps_b.tile([128, TILE], FP32, tag="pb", name="CX")
        nc.tensor.matmul(out=CX[:], lhsT=c_bf[:], rhs=embT[:], start=True, stop=True)
        cx_sb = act_p.tile([128, TILE], BF16, tag="cx_sb")
        nc.scalar.activation(cx_sb[:], CX[:], AF.Copy)
        XCX = act_p.tile([128, TILE], BF16, tag="XCX")
        nc.vector.tensor_tensor(out=XCX[:], in0=embT[:], in1=cx_sb[:], op=AX.mult)
        return {"embT": embT, "XCX": XCX}

    def layer(rhs_parts, Wc, bp, Dout, tagb, relu_engines):
        outs = []
        n_m = (Dout + 127) // 128
        psums = []
        for mi in range(n_m):
            mm = min(128, Dout - mi * 128)
            if mi == 0:
                P = ps_a.tile([mm, TILE], FP32, tag="pa", name=f"{tagb}{mi}")
            else:
                P = ps_b.tile([mm, TILE], FP32, tag="pb", name=f"{tagb}{mi}")
            nk = len(rhs_parts)
            for ki in range(nk):
                Wt = next(c[4] for c in Wc if c[0] == ki and c[1] == mi)
                nc.tensor.matmul(
                    out=P[:],
                    lhsT=Wt[:],
                    rhs=rhs_parts[ki][:],
                    start=(ki == 0),
                    stop=(ki == nk - 1),
                )
            psums.append((mi, mm, P))
        for mi, mm, P in psums:
            o = act_p.tile([mm, TILE], BF16, tag=f"{tagb}o{mi}")
            if relu_engines[mi] == "act":
                nc.scalar.activation(o[:], P[:], AF.Relu, bias=bp[mi][:])
            else:
                nc.vector.tensor_scalar(
                    out=o[:],
                    in0=P[:],
                    scalar1=bp[mi][:],
                    scalar2=0.0,
                    op0=AX.add,
                    op1=AX.max,
                )
            outs.append(o)
        return outs

    def stage_b(st, T, a):
        embT, XCX = a["embT"], a["XCX"]
        h1 = layer([embT], W1c, b1p, 200, "H1", ("vec", "act"))
        h2 = layer(h1, W2c, b2p, 256, "H2", ("act", "vec"))
        h3 = layer(h2, W3c, b3p, 200, "H3", ("vec", "act"))
        h4 = layer(h3, W4c, b4p, 128, "H4", ("vec",))
        STB = ps_s.tile([128, 4], FP32, tag="ps", name="STB")
        for j in range(4):
            cj = slice(j * 128, (j + 1) * 128)
            nc.tensor.matmul(
                out=STB[:, j : j + 1], lhsT=XCX[:, cj], rhs=ones1[:],
                start=True, stop=False,
            )
            nc.tensor.matmul(
                out=STB[:, j : j + 1], lhsT=h4[0][:, cj], rhs=W5b[:],
                start=False, stop=True,
            )
        nc.vector.tensor_tensor(
            out=st["argt"][:, 4 * T : 4 * T + 4],
            in0=STB[:],
            in1=st["LIN"][:, 4 * T : 4 * T + 4],
            op=AX.add,
        )

    def finish_macro(st):
        nc.scalar.activation(st["yt"][:], st["argt"][:], AF.Sigmoid, bias=c0b[:])
        nc.sync.dma_start(
            bass.AP(y.tensor, st["s0"], [[K_PER_P, 128], [1, K_PER_P]]),
            st["yt"][:],
        )

    flat = [(i, T) for i in range(n_macros * passes) for T in range(4)]
    a_results: dict = {}
    for i in range(-LOOKAHEAD, len(flat)):
        if i + LOOKAHEAD < len(flat):
            ia, TA = flat[i + LOOKAHEAD]
            if ia not in macro_state:
                macro_state[ia] = setup_macro(ia % n_macros)
            a_results[(ia, TA)] = stage_a(macro_state[ia], TA)
        if i >= 0:
            ib, TB = flat[i]
            stage_b(macro_state[ib], TB, a_results.pop((ib, TB)))
            if TB == 3:
                finish_macro(macro_state.pop(ib))


def build_nc(
    S: int = 32768,
    num_devices: int = 8,
    enable_asserts: bool = False,
    passes: int = 1,
):
    nc = bacc.Bacc(
        "TRN2",
        target_bir_lowering=False,
        debug=False,
        enable_asserts=enable_asserts,
        num_devices=num_devices,
    )
    aps = {}
    specs = [
        ("user_idx", [S], I32),
        ("item_idx", [S], I32),
        ("tab", [2 * HASH_BINS, TABW], U16),
        ("lin_b", [1], FP32),
        ("cross_k", [128, 128], FP32),
        ("W1", [128, 200], FP32),
        ("b1", [200], FP32),
        ("W2", [200, 256], FP32),
        ("b2", [256], FP32),
        ("W3", [256, 200], FP32),
        ("b3", [200], FP32),
        ("W4", [200, 128], FP32),
        ("b4", [128], FP32),
        ("W5", [128, 1], FP32),
        ("b5", [1], FP32),
    ]
    for name, shape, dt in specs:
        aps[name] = nc.dram_tensor(name, shape, dt, kind="ExternalInput").ap()
    aps["y"] = nc.dram_tensor("y", [S], FP32, kind="ExternalOutput").ap()

    with tile.TileContext(nc) as tc:
        with ExitStack() as ctx:
            emit_dfm(ctx, tc, aps, S, passes=passes)
    nc.compile()
    return nc


from concourse.bass_utils import run_bass_kernel_spmd

def _to_bf16(a: np.ndarray) -> np.ndarray:
    """fp32 -> bf16 (round-to-nearest-even) as a uint16-backed ml_dtypes array."""
    import ml_dtypes
    return a.astype(ml_dtypes.bfloat16)


B_FULL = 262144
N_CORES = 8
S_CORE = B_FULL // N_CORES

_NC_CACHE: dict = {}


def make_in_maps(inputs: dict) -> list:
    ue = np.asarray(inputs["user_emb"], dtype=np.float32)
    ie = np.asarray(inputs["item_emb"], dtype=np.float32)
    lw = np.asarray(inputs["lin_w"], dtype=np.float32)
    tab = np.empty((2 * HASH_BINS, TABW), np.float32)  # built fp32, cast below
    tab[:HASH_BINS, 0] = lw[:HASH_BINS]        # user rows: [lin | emb]
    tab[:HASH_BINS, 1:] = ue
    tab[HASH_BINS:, :EMB] = ie                 # item rows: [emb | lin]
    tab[HASH_BINS:, EMB] = lw[HASH_BINS:]
    shared = {"tab": np.ascontiguousarray(_to_bf16(tab).view(np.uint16))}
    for k in ("lin_b", "cross_k", "W1", "b1", "W2", "b2", "W3", "b3",
              "W4", "b4", "W5", "b5"):
        shared[k] = np.ascontiguousarray(np.asarray(inputs[k], dtype=np.float32))
    u = np.ascontiguousarray(np.asarray(inputs["user_idx"], dtype=np.int32))
    i = np.ascontiguousarray(np.asarray(inputs["item_idx"], dtype=np.int32))
    in_maps = []
    for c in range(N_CORES):
        m = dict(shared)
        m["user_idx"] = u[c * S_CORE : (c + 1) * S_CORE]
        m["item_idx"] = i[c * S_CORE : (c + 1) * S_CORE]
        in_maps.append(m)
    return in_maps


def kernel(**inputs) -> np.ndarray:
    if "nc" not in _NC_CACHE:
        _NC_CACHE["nc"] = build_nc(S_CORE, num_devices=N_CORES)
    nc = _NC_CACHE["nc"]
    in_maps = make_in_maps(inputs)
    cores = list(range(N_CORES))
    if "primed" not in _NC_CACHE:
        # The very first execution of a freshly loaded NEFF occasionally
        # returns bad gather data (unprimed SWDGE state on this runtime);
        # discard one execution before the real one.
        run_bass_kernel_spmd(nc, in_maps, core_ids=cores)
        _NC_CACHE["primed"] = True
    res = run_bass_kernel_spmd(nc, in_maps, core_ids=cores)
    y = np.concatenate([res.results[c]["y"] for c in range(N_CORES)])
    return y.reshape(B_FULL, 1).astype(np.float32)
